# revision 34
# baseline (speedup 1.0000x reference)
"""MoD (mixture-of-depths) attention Bass kernel for Trainium2, 8 NeuronCores.

Problem: B=4, L=4096, D=1024, H=16, HD=64, K=1024 (top-25% tokens per row).
  scores = x @ w_router + b ; idx = top_k(scores, 1024) (desc order)
  xs = x[idx]; causal attention over score-ordered subsequence; out = x with
  selected rows replaced by attention output.

Split of work:
  Host: router scores (fp32 matvec), top-k + descending ordering, gather of
  the K selected rows, transpose/tiling into the exact SBUF layouts (bf16),
  final scatter + pair-sum. These are selection/layout ops — cheap on host,
  expensive on device — and doing them here removes all gpsimd custom ops,
  indirect DMAs and collectives from the device program while cutting the
  staged bytes from ~256MB to ~40MB.

  Device (8 cores, no collectives): core pair (2b, 2b+1) handles batch row b;
  within a pair the 16 heads are split 8/8. Each core runs a dense pipeline in
  bf16 (fp32 PSUM accumulate): V/Q/K projections, causal attention over the
  score-ordered subsequence (S^T tiles [128tk, 512tq], exp on the scalar
  engine, 0/1 causal mask multiply on the vector engine, softmax denominator
  via an extra ones-column in V, normalization via a reciprocal outer-product
  matmul), then a partial out-projection over its 512 e-dims. Host adds the
  two partials and scatters: out[b] = x[b].copy(); out[b][idx] = yA + yB.
"""

import os

os.environ.setdefault("JAX_PLATFORMS", "axon,cpu")

import numpy as np
import ml_dtypes

import concourse.bass as bass
import concourse.bacc as bacc
import concourse.mybir as mybir
import concourse.tile as tile

F32 = mybir.dt.float32
BF16 = mybir.dt.bfloat16
AF = mybir.ActivationFunctionType
OP = mybir.AluOpType
BF = ml_dtypes.bfloat16

B, L, D = 4, 4096, 1024
H, HD = 16, 64
K = 1024
SCALE = 1.0 / 8.0
EH = 512          # e-dims per core (8 heads)
N_TC = 8          # token chunks of 128 (K = 1024)
N_EBLK = 4        # e-blocks of 128 per core


def _masks():
    # 0/1 causal masks for S^T tiles [tk=128, tq=512]: tile (m, n) crossing
    # the diagonal has di = m - 4n in {0,1,2,3}; entry (p, f) is valid iff
    # tk <= tq i.e. p + di*128 <= f.
    p = np.arange(128)[:, None]
    f = np.arange(512)[None, :]
    m = np.zeros((4, 128, 512), dtype=BF)
    for di in range(4):
        m[di] = (p + di * 128 <= f).astype(BF)
    return m


def build_program(n_cores=8, percore_shapes=False):
    """Builds the SPMD Bass program (same program on all cores; per-core
    behavior comes only from per-core input data). The program is
    collective-free, so the n_cores=1 build is identical in structure and
    is used for TimelineSim."""
    nc = bacc.Bacc("TRN2", num_devices=n_cores, debug=False)

    # ---- I/O (bf16, pre-tiled host-side into exact SBUF layouts) ----
    # All inputs are packed into ONE flat blob per core so the host pays a
    # single large tunnel transfer instead of five small ones. Layout:
    #   xsT  [128, 8, 1024]      xsT[p, dblk, t] = xs[t, dblk*128 + p]
    #   wq   [128, 4, 8, 128]    wq[p, eblk, dblk, e'] (half of e-dims)
    #   wk   [128, 4, 8, 128]
    #   wv   [128, 8, 512]       wv[p, dblk, e]
    #   wo   [128, 4, 1024]      wo[p(=e'), eblk, d]
    SZ_XST = 128 * 8 * K
    SZ_WQK = 128 * N_EBLK * 8 * 128
    SZ_WV = 128 * 8 * EH
    SZ_WO = 128 * N_EBLK * D
    TOT = SZ_XST + 2 * SZ_WQK + SZ_WV + SZ_WO
    blob = nc.dram_tensor("blob", [TOT], BF16, kind="ExternalInput")
    o0 = 0
    xsT_in = blob[o0 : o0 + SZ_XST].rearrange("(p a t) -> p a t", p=128, a=8)
    o0 += SZ_XST
    wq_in = blob[o0 : o0 + SZ_WQK].rearrange(
        "(p e a f) -> p e a f", p=128, e=N_EBLK, a=8
    )
    o0 += SZ_WQK
    wk_in = blob[o0 : o0 + SZ_WQK].rearrange(
        "(p e a f) -> p e a f", p=128, e=N_EBLK, a=8
    )
    o0 += SZ_WQK
    wv_in = blob[o0 : o0 + SZ_WV].rearrange("(p a f) -> p a f", p=128, a=8)
    o0 += SZ_WV
    wo_in = blob[o0 : o0 + SZ_WO].rearrange("(p e f) -> p e f", p=128, e=N_EBLK)
    y_out = nc.dram_tensor("y_out", [K, D], BF16, kind="ExternalOutput")

    m01 = nc.inline_tensor(_masks(), name="c_m01")

    with tile.TileContext(nc) as tc:
        with (
            tc.tile_pool(name="act", bufs=1) as actp,
            tc.tile_pool(name="wts", bufs=1) as wp,
            tc.tile_pool(name="es", bufs=12) as esp,
            tc.tile_pool(name="sm", bufs=2) as smp,
            tc.tile_pool(name="ys", bufs=6) as ysp,
            tc.tile_pool(name="psS", bufs=5, space="PSUM") as psS,
            tc.tile_pool(name="psPO", bufs=2, space="PSUM") as psPO,
            tc.tile_pool(name="psR", bufs=1, space="PSUM") as psR,
        ):
            # ---------- constants + inputs to SBUF ----------
            # order matters: the V phase needs only wv + the first token
            # quarter of xsT, so those transfer first and compute starts
            # ~6us earlier than a monolithic load.
            wv_sb = wp.tile([128, 8, EH], BF16)
            nc.sync.dma_start(wv_sb[:], wv_in)
            xsT = actp.tile([128, 8, K], BF16)
            for q in range(4):
                qsl = bass.ts(q, 256)
                nc.sync.dma_start(xsT[:, :, qsl], xsT_in[:, :, qsl])
            wq_sb = wp.tile([128, N_EBLK, 8, 128], BF16)
            nc.sync.dma_start(wq_sb[:], wq_in)
            wk_sb = wp.tile([128, N_EBLK, 8, 128], BF16)
            nc.sync.dma_start(wk_sb[:], wk_in)
            masks = wp.tile([128, 4, 512], BF16)
            for di in range(4):
                nc.sync.dma_start(masks[:, di, :], m01[di])
            wo_sb = wp.tile([128, N_EBLK, D], BF16)
            nc.sync.dma_start(wo_sb[:], wo_in)

            ones_bf = wp.tile([1, 64], BF16)
            nc.vector.memset(ones_bf[:], 1.0)

            # ---------- V [tc][128t, 8h, 65] (col 64 = ones for denom) ----------
            v_sb = actp.tile([128, N_TC, 8, 65], BF16)
            v_one = wp.tile([128, N_TC * 8], BF16)
            nc.vector.memset(v_one[:], 1.0)
            nc.vector.tensor_copy(
                v_sb[:, :, :, 64], v_one[:].rearrange("p (t h) -> p t h", t=N_TC)
            )
            for t in range(N_TC):
                pv = psS.tile([128, 512], F32, tag="ps")
                for dblk in range(8):
                    nc.tensor.matmul(
                        pv[:],
                        xsT[:, dblk, t * 128 : (t + 1) * 128],
                        wv_sb[:, dblk, :],
                        start=(dblk == 0), stop=(dblk == 7),
                    )
                # ACT is idle during the V phase
                nc.scalar.activation(
                    v_sb[:, t, :, 0:64],
                    pv[:].rearrange("p (h e) -> p h e", h=8),
                    AF.Copy,
                )

            # ---------- Q^T/K^T [128e, 1024t] + attention, software-pipelined ----
            qT = actp.tile([128, N_EBLK, K], BF16)
            kT = actp.tile([128, N_EBLK, K], BF16)
            oT = actp.tile([128, N_EBLK, K], BF16)

            def qk_unit(eblk, tch, w_sb, dst):
                # one 8-matmul projection chain + its PSUM->SBUF(bf16) copy
                tsl = bass.ts(tch, 512)
                ps = psS.tile([128, 512], F32, tag="ps")
                for dblk in range(8):
                    nc.tensor.matmul(
                        ps[:], w_sb[:, eblk, dblk, :], xsT[:, dblk, tsl],
                        start=(dblk == 0), stop=(dblk == 7),
                    )
                nc.vector.tensor_copy(dst[:, eblk, tsl], ps[:])

            def qk_units(eblk):
                return [
                    (eblk, tch, w, d)
                    for tch in range(2)
                    for (w, d) in ((wq_sb, qT), (wk_sb, kT))
                ]

            # spread DMAs across engine queues so DGE setup parallelizes
            dma_queues = [nc.sync, nc.scalar]

            def outproj_unit(t, dc):
                py = psS.tile([128, 512], F32, tag="ps")
                for eblk in range(N_EBLK):
                    nc.tensor.matmul(
                        py[:],
                        oT[:, eblk, t * 128 : (t + 1) * 128],
                        wo_sb[:, eblk, dc * 512 : (dc + 1) * 512],
                        start=(eblk == 0), stop=(eblk == N_EBLK - 1),
                    )
                y_sb = ysp.tile([128, 512], BF16, tag="ysb")
                nc.vector.tensor_copy(y_sb[:], py[:])
                dma_queues[(2 * t + dc) % 2].dma_start(
                    y_out[t * 128 : (t + 1) * 128, dc * 512 : (dc + 1) * 512],
                    y_sb[:],
                )

            # queue of independent PE chain emitters, pumped mid-attention so
            # the PE always has work while exp/mask streams catch up
            filler = []

            def pump():
                if filler:
                    filler.pop(0)()

            # normalization for a finished block is deferred into the NEXT
            # block so its reciprocal latency never blocks the PE stream
            pending_norm = []

            def normalize(eblk, sub, n, po):
                esl = slice(sub * 64, sub * 64 + 64)
                tql = bass.ts(n, 512)
                # bf16 reciprocal (0.4% rounding) is well within the
                # tolerance budget and keeps the broadcast outer product at
                # 1 cycle/row.
                r_row = smp.tile([1, 512], BF16, tag="rr")
                with nc.allow_low_precision(reason="softmax denom bf16"):
                    nc.vector.reciprocal(r_row[:], po[64:65, :])
                r_bc = psR.tile([64, 512], F32, tag="rbc")
                nc.tensor.matmul(
                    r_bc[:], ones_bf[:], r_row[:], start=True, stop=True,
                )
                # vector ops may read only one PSUM operand: stage the
                # broadcast reciprocal in SBUF before the multiply
                r_sb = smp.tile([64, 512], BF16, tag="rsb")
                nc.vector.tensor_copy(r_sb[:], r_bc[:])
                nc.vector.tensor_tensor(
                    out=oT[esl, eblk, tql],
                    in0=po[0:64, :], in1=r_sb[:], op=OP.mult,
                )

            def flush_norm():
                while pending_norm:
                    pending_norm.pop(0)()

            def attn_block(eblk, sub, n, pumps=1):
                hh = eblk * 2 + sub
                esl = slice(sub * 64, sub * 64 + 64)
                po = psPO.tile([65, 512], F32, tag="po")
                n_m = 4 * n + 4
                es_tiles = []

                def s_tile(m):
                    # a diagonal-crossing tile (di >= 0) has its first di*128
                    # columns fully masked for every partition: restrict
                    # S/exp/mask/PV to the live columns.
                    di = m - 4 * n
                    lo = di * 128 if di > 0 else 0
                    csl = slice(lo, 512)
                    ps_s = psS.tile([128, 512], F32, tag="ps")
                    nc.tensor.matmul(
                        ps_s[:, csl],
                        kT[esl, eblk, m * 128 : (m + 1) * 128],
                        qT[esl, eblk, n * 512 + lo : (n + 1) * 512],
                        start=True, stop=True,
                        tile_position=(sub * 64, 0),
                    )
                    es = esp.tile([128, 512], BF16, tag="es")
                    nc.scalar.activation(
                        es[:, csl], ps_s[:, csl], AF.Exp, scale=SCALE
                    )
                    if di >= 0:
                        # SBUF-only multiply: wide masks go to the otherwise-
                        # idle gpsimd engine, narrow ones to the vector engine
                        eng = nc.gpsimd if di < 2 else nc.vector
                        eng.tensor_tensor(
                            out=es[:, csl], in0=es[:, csl],
                            in1=masks[:, di, csl], op=OP.mult,
                        )
                    es_tiles.append((es, csl))

                def pv_tile(m):
                    es, csl = es_tiles[m]
                    nc.tensor.matmul(
                        po[:, csl], v_sb[:, m, hh, :], es[:, csl],
                        start=(m == 0), stop=(m == n_m - 1),
                    )

                # S runs ~4 tiles ahead of PV so the PE never waits for the
                # exp/mask stream; the pump slots independent chain work in
                # the middle of the block.
                for m in range(4):
                    s_tile(m)
                flush_norm()
                pump()
                for m in range(4, n_m):
                    s_tile(m)
                    pv_tile(m - 4)
                if pumps > 1:
                    pump()
                for m in range(max(n_m - 4, 0), n_m):
                    pv_tile(m)
                pending_norm.append(lambda: normalize(eblk, sub, n, po))

            for u in qk_units(0):
                qk_unit(*u)

            for eblk in range(N_EBLK - 1):
                filler.extend(
                    (lambda u=u: qk_unit(*u)) for u in qk_units(eblk + 1)
                )
                for sub in range(2):
                    for n in range(2):
                        attn_block(eblk, sub, n)

            # last eblk: run both heads' n=0 blocks first so the t<512
            # out-projection columns unlock early, then interleave those
            # out-proj chains into the n=1 blocks.
            emitted = []

            def op_filler(t, dc):
                emitted.append((t, dc))
                outproj_unit(t, dc)

            attn_block(N_EBLK - 1, 0, 0)
            attn_block(N_EBLK - 1, 1, 0)
            filler.extend(
                (lambda t=t, dc=dc: op_filler(t, dc))
                for t in (0, 1, 2, 3) for dc in (0, 1)
            )
            attn_block(N_EBLK - 1, 0, 1, pumps=2)
            attn_block(N_EBLK - 1, 1, 1, pumps=2)
            flush_norm()

            # ---------- remaining out-projection partials ----------
            for t in range(N_TC):
                for dc in range(2):
                    if (t, dc) not in emitted:
                        outproj_unit(t, dc)

    nc.compile()
    return nc


_NC_CACHE = {}


def _get_nc(n_cores=8):
    if n_cores not in _NC_CACHE:
        _NC_CACHE[n_cores] = build_program(n_cores)
    return _NC_CACHE[n_cores]


_DISPATCH_CACHE = {}


def _get_dispatch(nc, n_cores=8):
    """SPMD dispatch tuned for the axon tunnel: per-device parallel
    device_put of the inputs (~3x the single-stream tunnel bandwidth),
    output donation buffers created on-device instead of uploading zeros,
    and a jit cached across kernel() calls. Mirrors the metadata handling
    of bass2jax.run_bass_via_pjrt, minus collectives/partition-id support
    (this program uses neither)."""
    key = id(nc)
    if key in _DISPATCH_CACHE:
        return _DISPATCH_CACHE[key]

    import jax
    import jax.numpy as jnp
    from jax.sharding import Mesh, NamedSharding, PartitionSpec
    from jax.experimental.shard_map import shard_map
    from concourse.bass2jax import (
        _bass_exec_p,
        install_neuronx_cc_hook,
        partition_id_tensor,
    )

    install_neuronx_cc_hook()
    assert nc.dbg_addr is None

    partition_name = (
        nc.partition_id_tensor.name if nc.partition_id_tensor else None
    )
    in_names, out_names, out_avals = [], [], []
    for alloc in nc.m.functions[0].allocations:
        if not isinstance(alloc, mybir.MemoryLocationSet):
            continue
        name = alloc.memorylocations[0].name
        if alloc.kind == "ExternalInput":
            if name != partition_name:
                in_names.append(name)
        elif alloc.kind == "ExternalOutput":
            out_names.append(name)
            out_avals.append(
                jax.core.ShapedArray(
                    tuple(alloc.tensor_shape), mybir.dt.np(alloc.dtype)
                )
            )
    n_params = len(in_names)
    all_names = list(in_names) + list(out_names)
    if partition_name is not None:
        all_names.append(partition_name)
    all_names = tuple(all_names)

    def _body(*args):
        operands = list(args)
        if partition_name is not None:
            operands.append(partition_id_tensor())
        return tuple(
            _bass_exec_p.bind(
                *operands,
                out_avals=tuple(out_avals),
                in_names=all_names,
                out_names=tuple(out_names),
                lowering_input_output_aliases=(),
                sim_require_finite=True,
                sim_require_nnan=True,
                nc=nc,
            )
        )

    devices = jax.devices()[:n_cores]
    mesh = Mesh(np.asarray(devices), ("core",))
    spec = NamedSharding(mesh, PartitionSpec("core"))
    donate = tuple(range(n_params, n_params + len(out_avals)))
    sharded = jax.jit(
        shard_map(
            _body,
            mesh=mesh,
            in_specs=(PartitionSpec("core"),) * (n_params + len(out_avals)),
            out_specs=(PartitionSpec("core"),) * len(out_names),
            check_rep=False,
        ),
        donate_argnums=donate,
        keep_unused=True,
    )

    def dispatch(in_maps):
        # parallel H2D: put every core's shard of every input, then stitch
        # the global arrays from the device-resident shards.
        shards = [
            [jax.device_put(np.asarray(in_maps[c][n]), devices[c]) for c in range(n_cores)]
            for n in in_names
        ]
        global_in = []
        for i, n in enumerate(in_names):
            sh = shards[i][0].shape
            global_in.append(
                jax.make_array_from_single_device_arrays(
                    (n_cores * sh[0], *sh[1:]), spec, shards[i]
                )
            )
        # donated output buffers: created on-device, nothing uploaded
        zero_shards = []
        for av in out_avals:
            zs = []
            for d in devices:
                with jax.default_device(d):
                    zs.append(jnp.zeros(av.shape, av.dtype))
            jax.block_until_ready(zs)
            zero_shards.append(
                jax.make_array_from_single_device_arrays(
                    (n_cores * av.shape[0], *av.shape[1:]), spec, zs
                )
            )
        out_arrs = sharded(*global_in, *zero_shards)
        out_np = [np.asarray(a) for a in out_arrs]
        return [
            {
                name: out_np[i].reshape(n_cores, *out_avals[i].shape)[c]
                for i, name in enumerate(out_names)
            }
            for c in range(n_cores)
        ]

    _DISPATCH_CACHE[key] = dispatch
    return dispatch


def _route_and_gather(x, w_router):
    """Host router: top-K indices per row (descending score, ties by index)
    and the gathered rows tiled to the device layout [128, 8dblk, K] bf16."""
    scores = x.reshape(-1, D) @ w_router  # bias shifts all scores equally;
    scores = scores.reshape(B, L)         # it cannot change the top-k or order
    idxs, xsTs = [], []
    for b in range(B):
        s = scores[b]
        part = np.argpartition(-s, K - 1)[:K]
        idx = part[np.lexsort((part, -s[part]))]
        idxs.append(idx)
        xsT = np.ascontiguousarray(x[b][idx].T)          # [D, K]
        xsT = xsT.reshape(8, 128, K).transpose(1, 0, 2)  # [p, dblk, t]
        xsTs.append(np.ascontiguousarray(xsT.astype(BF)))
    return idxs, xsTs


def _prep_weight_half(wq, wk, wv, wo, half):
    """Flat bf16 weight blob for one heads-half, in device layout order."""
    esl = slice(half * EH, (half + 1) * EH)
    wqh = wq[:, esl].reshape(8, 128, N_EBLK, 128).transpose(1, 2, 0, 3)
    wkh = wk[:, esl].reshape(8, 128, N_EBLK, 128).transpose(1, 2, 0, 3)
    wvh = wv[:, esl].reshape(8, 128, EH).transpose(1, 0, 2)
    woh = wo[esl, :].reshape(N_EBLK, 128, D).transpose(1, 0, 2)
    return np.concatenate(
        [
            wqh.astype(BF).ravel(),
            wkh.astype(BF).ravel(),
            wvh.astype(BF).ravel(),
            woh.astype(BF).ravel(),
        ]
    )


def kernel(x, w_router, b_router, wq, wk, wv, wo):
    x = np.ascontiguousarray(np.asarray(x, np.float32))
    w_router = np.asarray(w_router, np.float32).reshape(D)
    wq = np.asarray(wq, np.float32)
    wk = np.asarray(wk, np.float32)
    wv = np.asarray(wv, np.float32)
    wo = np.asarray(wo, np.float32)

    idxs, xsTs = _route_and_gather(x, w_router)
    halves = [_prep_weight_half(wq, wk, wv, wo, h) for h in range(2)]
    in_maps = [
        {"blob": np.concatenate([xsTs[c // 2].ravel(), halves[c % 2]])}
        for c in range(8)
    ]

    nc = _get_nc(8)
    results = _get_dispatch(nc, 8)(in_maps)

    out = x.copy()
    for b in range(B):
        ya = results[2 * b]["y_out"].astype(np.float32)
        yb = results[2 * b + 1]["y_out"].astype(np.float32)
        out[b][idxs[b]] = ya + yb
    return out
